# revision 19
# baseline (speedup 1.0000x reference)
"""Trainium2 Bass kernel for CrossModalFusion (B=4, C=64, H=W=64, N=4096).

Reference computation (per sample b, with x reshaped to [C, N]):
    q = wq @ xo + bq          [8, N]
    k = wk @ xs + bk          [8, N]
    v = wv @ xs + bv          [64, N]
    S[n, m]  = q[:, n] . k[:, m]
    attn     = softmax_m(S)
    out      = gamma * (v @ attn^T) + x_opt

Sharding: 8 cores = 4 batch samples x 2 halves of the query (n) axis.
Each core computes output rows [64, 2048] for its (sample, n-half); no
cross-core communication is needed.

Per-core dataflow (matmuls in bf16 / f32r — the PE in this environment never
leaves the 1.2 GHz throttled clock, so concurrency via PE array tiling is the
main lever):
  - biases are folded into augmented weights on the host (ones-row trick);
    gamma is folded into wv/bv on the host, so the attention output comes out
    pre-scaled and the softmax denominator column stays unscaled.
  - scores are computed TRANSPOSED (S^T[m, n]) so the exp'd scores feed the
    attention*V matmuls directly as the moving operand.  v^T gets an extra
    ones column, so the AV matmuls' row 64 accumulate sum_m exp(S[n, m]) —
    the softmax denominator for free.  No max-subtraction: scores are O(3).
  - q/k are replicated at partition offsets 0/32/64/96 so four rank-8 S^T
    matmuls run concurrently in the four 32-row PE groups.
  - AV matmuls are split into rows 0-63 / 64-127 (two concurrent 64-row PE
    groups) accumulating into separate PSUM tiles avA/avB, summed at
    normalize time.
  - q/k/vT prep is interleaved just-in-time into n-tile 0's wave loop so the
    exp pipeline starts as soon as the first score block exists.
  - per n-tile of 512: accumulate over all 32 m-blocks, normalize by
    1/denominator, add the fp32 x_opt residual, DMA out.
"""

import os
import sys

import numpy as np

for _p in ("/opt/trn_rl_repo", "/root/.axon_site/_ro/trn_rl_repo"):
    if os.path.isdir(_p) and _p not in sys.path:
        sys.path.insert(0, _p)

import concourse.bass as bass
import concourse.mybir as mybir
import concourse.tile as tile
from concourse import bacc
from concourse.bass_utils import run_bass_kernel_spmd

F32 = mybir.dt.float32
F32R = mybir.dt.float32r
BF16 = mybir.dt.bfloat16
AF = mybir.ActivationFunctionType

B, C, HH, WW = 4, 64, 64, 64
N = HH * WW            # 4096 key/query positions
D = 8                  # q/k channel count
CA = C + 1             # augmented channel dim (ones row / denominator row)
NCORES = 8
NL = N // 2            # query rows per core
NT = 512               # n-tile (PSUM bank width in fp32)
MB = 128               # m-block (PE partition width)
N_NT = NL // NT        # 4 n-tiles per core
N_MB = N // MB         # 32 m-blocks
E_DTYPE = F32R         # exp output / AV operand dtype


def build_program(repeat: int = 1) -> bass.Bass:
    nc = bacc.Bacc("TRN2", target_bir_lowering=False, num_devices=NCORES)
    xo_d = nc.declare_dram_parameter("xo_bf", [CA, NL], BF16, isOutput=False)
    xs_d = nc.declare_dram_parameter("xs_bf", [CA, N], BF16, isOutput=False)
    xr_d = nc.declare_dram_parameter("xores", [C, NL], F32, isOutput=False)
    wq_d = nc.declare_dram_parameter("wq_bf", [CA, 96 + D], BF16, isOutput=False)
    wk_d = nc.declare_dram_parameter("wk_bf", [CA, 96 + D], BF16, isOutput=False)
    wv_d = nc.declare_dram_parameter("wv_bf", [CA, CA], BF16, isOutput=False)
    out_d = nc.declare_dram_parameter("out", [C, NL], F32, isOutput=True)

    with tile.TileContext(nc) as tc:
      for _rep in range(repeat):
        with (
            tc.tile_pool(name="const", bufs=1) as cp,
            tc.tile_pool(name="st_ps", bufs=3, space="PSUM") as st_pool,
            tc.tile_pool(name="avA_ps", bufs=1, space="PSUM") as avA_pool,
            tc.tile_pool(name="avB_ps", bufs=1, space="PSUM") as avB_pool,
            tc.tile_pool(name="e_sb", bufs=4) as e_pool,
            tc.tile_pool(name="o_sb", bufs=2) as o_pool,
            tc.tile_pool(name="sm_sb", bufs=2) as sm_pool,
        ):
            xs_sb = cp.tile([CA, N], BF16)
            xo_sb = cp.tile([CA, NL], BF16)
            nc.sync.dma_start(xs_sb[:, 0:1024], xs_d[:, 0:1024])
            wk_sb = cp.tile([CA, 96 + D], BF16)
            nc.sync.dma_start(wk_sb[:], wk_d[:])
            wq_sb = cp.tile([CA, 96 + D], BF16)
            nc.sync.dma_start(wq_sb[:], wq_d[:])
            nc.sync.dma_start(xo_sb[:, 0:1024], xo_d[:, 0:1024])
            wv_sb = cp.tile([CA, CA], BF16)
            nc.sync.dma_start(wv_sb[:], wv_d[:])
            ones_sb = cp.tile([1, C], BF16)
            nc.vector.memset(ones_sb[:], 1.0)
            for j in range(1, 4):
                nc.sync.dma_start(
                    xs_sb[:, j * 1024 : (j + 1) * 1024],
                    xs_d[:, j * 1024 : (j + 1) * 1024],
                )
            nc.sync.dma_start(xo_sb[:, 1024:2048], xo_d[:, 1024:2048])
            xr_sb = cp.tile([C, NL], F32)
            nc.sync.dma_start(xr_sb[:], xr_d[:])

            # q/k replicated at partition offsets 0/32/64/96 (score row
            # groups); vT blocks [128, 65] with trailing ones column.
            q_rep = cp.tile([96 + D, NL], BF16)
            k_rep = cp.tile([96 + D, N], BF16)
            vT = cp.tile([MB, N_MB * CA], E_DTYPE)

            # wk_sb/wq_sb hold 4 copies of the weights at col offsets
            # 0/32/64/96, so one matmul lands k/q at all four partition
            # groups and one CAST moves them to SBUF -- no replication DMAs.
            def prep_k_chunk(c):
                kp = st_pool.tile([96 + D, NT], F32, tag="st", name=f"kp{c}")
                nc.tensor.matmul(
                    kp[:], wk_sb[:], xs_sb[:, c * NT : (c + 1) * NT],
                    start=True, stop=True,
                )
                nc.vector.tensor_copy(k_rep[:, c * NT : (c + 1) * NT], kp[:])

            def prep_q_chunk(c):
                qp = st_pool.tile([96 + D, NT], F32, tag="st", name=f"qp{c}")
                nc.tensor.matmul(
                    qp[:], wq_sb[:], xo_sb[:, c * NT : (c + 1) * NT],
                    start=True, stop=True,
                )
                nc.vector.tensor_copy(q_rep[:, c * NT : (c + 1) * NT], qp[:])

            def prep_vt_quad(p):
                # 4 vT blocks through one PSUM slot, one batched cast
                vp = st_pool.tile([MB, 4 * CA], F32, tag="st", name=f"vp{p}")
                for i in range(4):
                    mb = 4 * p + i
                    nc.tensor.matmul(
                        vp[:, i * CA : (i + 1) * CA],
                        xs_sb[:, mb * MB : (mb + 1) * MB], wv_sb[:],
                        start=True, stop=True,
                    )
                nc.vector.tensor_copy(vT[:, 4 * p * CA : (4 * p + 4) * CA], vp[:])

            prep_k_chunk(0)
            prep_q_chunk(0)

            pending_norm = []
            norm_state = {}

            def norm_a(nt, avA, avB):
                # DVE-only half: sum the split accumulators, reciprocal
                avAs = o_pool.tile([CA, NT], F32, tag="avAs", name=f"avAs{nt}")
                nc.vector.tensor_copy(avAs[:], avA[:])
                avS = o_pool.tile([CA, NT], F32, tag="avS", name=f"avS{nt}")
                nc.vector.tensor_add(avS[:], avB[:], avAs[:])
                recip = sm_pool.tile([1, NT], F32, tag="recip", name=f"recip{nt}")
                nc.vector.reciprocal(recip[:], avS[C:CA, :])
                recip_bf = sm_pool.tile([1, NT], BF16, tag="recip_bf", name=f"rb{nt}")
                nc.vector.tensor_copy(recip_bf[:], recip[:])
                norm_state[nt] = (avS, recip_bf)

            def norm_b(nt, avA, avB):
                avS, recip_bf = norm_state.pop(nt)
                n0b, n1b = nt * NT, (nt + 1) * NT
                bc = st_pool.tile([C, NT], F32, tag="st", name=f"bc{nt}")
                nc.tensor.matmul(bc[:], ones_sb[:], recip_bf[:], start=True, stop=True)
                om = o_pool.tile([C, NT], F32, tag="om", name=f"om{nt}")
                nc.vector.tensor_mul(om[:], bc[:], avS[0:C, :])
                o = o_pool.tile([C, NT], F32, tag="o", name=f"o{nt}")
                nc.vector.tensor_add(o[:], om[:], xr_sb[:, n0b:n1b])
                nc.sync.dma_start(out_d[:, n0b:n1b], o[:])

            for nt in range(N_NT):
                n0, n1 = nt * NT, (nt + 1) * NT
                avA = avA_pool.tile([CA, NT], F32, tag="avA", name=f"avA{nt}")
                avB = avB_pool.tile([CA, NT], F32, tag="avB", name=f"avB{nt}")

                def emit_av(e_t, w, avA=avA, avB=avB):
                    for j in range(2):
                        mb = 2 * w + j
                        nc.tensor.matmul(
                            avA[:],
                            vT[0:64, mb * CA : (mb + 1) * CA],
                            e_t[0:64, j * NT : (j + 1) * NT],
                            start=(mb == 0), stop=(mb == N_MB - 1),
                        )
                        nc.tensor.matmul(
                            avB[:],
                            vT[64:MB, mb * CA : (mb + 1) * CA],
                            e_t[64:MB, j * NT : (j + 1) * NT],
                            start=(mb == 0), stop=(mb == N_MB - 1),
                        )

                pend = []
                for p in range(N_MB // 4):  # wave pairs: m-blocks 4p..4p+3
                    # score quad: 4 concurrent rank-8 matmuls in distinct
                    # 32-row PE groups
                    st0 = st_pool.tile([MB, 2 * NT], F32, tag="st", name=f"st{nt}_{p}a")
                    st1 = st_pool.tile([MB, 2 * NT], F32, tag="st", name=f"st{nt}_{p}b")
                    for j4 in range(4):
                        mb = 4 * p + j4
                        rg = 32 * j4
                        stt, col = (st0, j4 * NT) if j4 < 2 else (st1, (j4 - 2) * NT)
                        nc.tensor.matmul(
                            stt[:, col : col + NT],
                            k_rep[rg : rg + D, mb * MB : (mb + 1) * MB],
                            q_rep[rg : rg + D, n0:n1],
                            start=True, stop=True,
                            tile_position=(rg, 0),
                        )
                    e0 = e_pool.tile([MB, 2 * NT], E_DTYPE, tag="e", name=f"e{nt}_{p}a")
                    nc.scalar.activation(e0[:], st0[:], AF.Exp)
                    e1 = e_pool.tile([MB, 2 * NT], E_DTYPE, tag="e", name=f"e{nt}_{p}b")
                    nc.scalar.activation(e1[:], st1[:], AF.Exp)
                    if pending_norm and p == 0:
                        norm_a(*pending_norm[0])
                    if pending_norm and p == 3:
                        norm_b(*pending_norm.pop(0))
                    for args in pend:
                        emit_av(*args)
                    if nt == 0:
                        if p + 1 < 8:
                            prep_k_chunk(p + 1)
                        prep_vt_quad(p)
                    if nt < N_NT - 1 and p == 1:
                        # prep the NEXT tile's q chunk here, spreading the
                        # prep load that used to pile onto n-tile 0
                        prep_q_chunk(nt + 1)
                    pend = [(e0, 2 * p), (e1, 2 * p + 1)]
                for args in pend:
                    emit_av(*args)

                pending_norm.append((nt, avA, avB))
                if nt == N_NT - 1:
                    while pending_norm:
                        norm_a(*pending_norm[0])
                        norm_b(*pending_norm.pop(0))
    nc.compile()
    return nc


_NC = None


def _get_nc() -> bass.Bass:
    global _NC
    if _NC is None:
        _NC = build_program()
    return _NC


def _to_bf16(a: np.ndarray) -> np.ndarray:
    """Round-to-nearest-even fp32 -> bf16 (ml_dtypes view)."""
    import ml_dtypes

    u = np.ascontiguousarray(a, np.float32).view(np.uint32)
    rounded = ((u + 0x7FFF + ((u >> 16) & 1)) >> 16).astype(np.uint16)
    return rounded.view(ml_dtypes.bfloat16)


def make_in_maps(x_opt, x_sar, wq, bq, wk, bk, wv, bv, gamma):
    f = np.float32
    x_opt = np.asarray(x_opt, f).reshape(B, C, N)
    x_sar = np.asarray(x_sar, f).reshape(B, C, N)
    g = float(np.asarray(gamma, f).reshape(()))
    wq_aug = np.concatenate([np.asarray(wq, f).T, np.asarray(bq, f)[None, :]], axis=0)
    wk_aug = np.concatenate([np.asarray(wk, f).T, np.asarray(bk, f)[None, :]], axis=0)
    # gamma folded into v (weights AND bias); denominator column stays 1.
    wv_aug = np.zeros((CA, CA), f)
    wv_aug[:C, :C] = np.asarray(wv, f).T * g
    wv_aug[C, :C] = np.asarray(bv, f) * g
    wv_aug[C, C] = 1.0
    wq4 = np.zeros((CA, 96 + D), f)
    wk4 = np.zeros((CA, 96 + D), f)
    for gidx in range(4):
        wq4[:, 32 * gidx : 32 * gidx + D] = wq_aug
        wk4[:, 32 * gidx : 32 * gidx + D] = wk_aug
    wq_bf = _to_bf16(wq4)
    wk_bf = _to_bf16(wk4)
    wv_bf = _to_bf16(wv_aug)
    ones_n = np.ones((1, N), f)
    maps = []
    for core in range(NCORES):
        b, h = divmod(core, 2)
        xo_aug = np.concatenate(
            [x_opt[b, :, h * NL : (h + 1) * NL], ones_n[:, :NL]], axis=0
        )
        xs_aug = np.concatenate([x_sar[b], ones_n], axis=0)
        maps.append(
            {
                "xo_bf": _to_bf16(xo_aug),
                "xs_bf": _to_bf16(xs_aug),
                "xores": np.ascontiguousarray(x_opt[b, :, h * NL : (h + 1) * NL]),
                "wq_bf": wq_bf,
                "wk_bf": wk_bf,
                "wv_bf": wv_bf,
            }
        )
    return maps


def assemble_out(results) -> np.ndarray:
    out = np.empty((B, C, N), np.float32)
    for core in range(NCORES):
        b, h = divmod(core, 2)
        out[b, :, h * NL : (h + 1) * NL] = results[core]["out"]
    return out.reshape(B, C, HH, WW)


def kernel(**inputs) -> np.ndarray:
    nc = _get_nc()
    maps = make_in_maps(**inputs)
    res = run_bass_kernel_spmd(nc, maps, list(range(NCORES)))
    return assemble_out(res.results)


# revision 21
# speedup vs baseline: 1.0331x; 1.0331x over previous
"""Trainium2 Bass kernel for CrossModalFusion (B=4, C=64, H=W=64, N=4096).

Reference computation (per sample b, with x reshaped to [C, N]):
    q = wq @ xo + bq          [8, N]
    k = wk @ xs + bk          [8, N]
    v = wv @ xs + bv          [64, N]
    S[n, m]  = q[:, n] . k[:, m]
    attn     = softmax_m(S)
    out      = gamma * (v @ attn^T) + x_opt

Sharding: 8 cores = 4 batch samples x 2 halves of the query (n) axis.
Each core computes output rows [64, 2048] for its (sample, n-half); no
cross-core communication is needed.

Per-core dataflow (matmuls in bf16 / f32r — the PE in this environment never
leaves the 1.2 GHz throttled clock, so concurrency via PE array tiling is the
main lever):
  - biases are folded into augmented weights on the host (ones-row trick);
    gamma is folded into wv/bv on the host, so the attention output comes out
    pre-scaled and the softmax denominator column stays unscaled.
  - scores are computed TRANSPOSED (S^T[m, n]) so the exp'd scores feed the
    attention*V matmuls directly as the moving operand.  v^T gets an extra
    ones column, so the AV matmuls' row 64 accumulate sum_m exp(S[n, m]) —
    the softmax denominator for free.  No max-subtraction: scores are O(3).
  - q/k are replicated at partition offsets 0/32/64/96 so four rank-8 S^T
    matmuls run concurrently in the four 32-row PE groups.
  - AV matmuls are split into rows 0-63 / 64-127 (two concurrent 64-row PE
    groups) accumulating into separate PSUM tiles avA/avB, summed at
    normalize time.
  - q/k/vT prep is interleaved just-in-time into n-tile 0's wave loop so the
    exp pipeline starts as soon as the first score block exists.
  - per n-tile of 512: accumulate over all 32 m-blocks, normalize by
    1/denominator, add the fp32 x_opt residual, DMA out.
"""

import os
import sys

import numpy as np

for _p in ("/opt/trn_rl_repo", "/root/.axon_site/_ro/trn_rl_repo"):
    if os.path.isdir(_p) and _p not in sys.path:
        sys.path.insert(0, _p)

import concourse.bass as bass
import concourse.mybir as mybir
import concourse.tile as tile
from concourse import bacc
from concourse.bass_utils import run_bass_kernel_spmd

F32 = mybir.dt.float32
F32R = mybir.dt.float32r
BF16 = mybir.dt.bfloat16
AF = mybir.ActivationFunctionType

B, C, HH, WW = 4, 64, 64, 64
N = HH * WW            # 4096 key/query positions
D = 8                  # q/k channel count
CA = C + 1             # augmented channel dim (ones row / denominator row)
NCORES = 8
NL = N // 2            # query rows per core
NT = 512               # n-tile (PSUM bank width in fp32)
MB = 128               # m-block (PE partition width)
N_NT = NL // NT        # 4 n-tiles per core
N_MB = N // MB         # 32 m-blocks
E_DTYPE = F32R         # exp output / AV operand dtype


def build_program(repeat: int = 1) -> bass.Bass:
    nc = bacc.Bacc("TRN2", target_bir_lowering=False, num_devices=NCORES)
    xo_d = nc.declare_dram_parameter("xo_bf", [CA, NL], BF16, isOutput=False)
    xs_d = nc.declare_dram_parameter("xs_bf", [CA, N], BF16, isOutput=False)
    xr_d = nc.declare_dram_parameter("xores", [C, NL], F32, isOutput=False)
    wq_d = nc.declare_dram_parameter("wq_bf", [CA, 96 + D], BF16, isOutput=False)
    wk_d = nc.declare_dram_parameter("wk_bf", [CA, 96 + D], BF16, isOutput=False)
    wv_d = nc.declare_dram_parameter("wv_bf", [CA, CA], BF16, isOutput=False)
    out_d = nc.declare_dram_parameter("out", [C, NL], F32, isOutput=True)

    with tile.TileContext(nc) as tc:
      for _rep in range(repeat):
        with (
            tc.tile_pool(name="const", bufs=1) as cp,
            tc.tile_pool(name="st_ps", bufs=3, space="PSUM") as st_pool,
            tc.tile_pool(name="avA_ps", bufs=1, space="PSUM") as avA_pool,
            tc.tile_pool(name="avB_ps", bufs=1, space="PSUM") as avB_pool,
            tc.tile_pool(name="e_sb", bufs=4) as e_pool,
            tc.tile_pool(name="o_sb", bufs=2) as o_pool,
            tc.tile_pool(name="sm_sb", bufs=2) as sm_pool,
        ):
            xs_sb = cp.tile([CA, N], BF16)
            xo_sb = cp.tile([CA, NL], BF16)
            nc.sync.dma_start(xs_sb[:, 0:1024], xs_d[:, 0:1024])
            wk_sb = cp.tile([CA, 96 + D], BF16)
            nc.sync.dma_start(wk_sb[:], wk_d[:])
            wq_sb = cp.tile([CA, 96 + D], BF16)
            nc.sync.dma_start(wq_sb[:], wq_d[:])
            nc.sync.dma_start(xo_sb[:, 0:1024], xo_d[:, 0:1024])
            wv_sb = cp.tile([CA, CA], BF16)
            nc.sync.dma_start(wv_sb[:], wv_d[:])
            ones_sb = cp.tile([1, C], BF16)
            nc.vector.memset(ones_sb[:], 1.0)
            for j in range(1, 4):
                nc.sync.dma_start(
                    xs_sb[:, j * 1024 : (j + 1) * 1024],
                    xs_d[:, j * 1024 : (j + 1) * 1024],
                )
            nc.sync.dma_start(xo_sb[:, 1024:2048], xo_d[:, 1024:2048])
            xr_sb = cp.tile([C, NL], F32)
            nc.sync.dma_start(xr_sb[:], xr_d[:])

            # q/k replicated at partition offsets 0/32/64/96 (score row
            # groups); vT blocks [128, 65] with trailing ones column.
            q_rep = cp.tile([96 + D, NL], BF16)
            k_rep = cp.tile([96 + D, N], BF16)
            vT = cp.tile([MB, N_MB * CA], E_DTYPE)

            # wk_sb/wq_sb hold 4 copies of the weights at col offsets
            # 0/32/64/96, so one matmul lands k/q at all four partition
            # groups and one CAST moves them to SBUF -- no replication DMAs.
            def prep_k_chunk(c):
                kp = st_pool.tile([96 + D, NT], F32, tag="st", name=f"kp{c}")
                nc.tensor.matmul(
                    kp[:], wk_sb[:], xs_sb[:, c * NT : (c + 1) * NT],
                    start=True, stop=True,
                )
                nc.vector.tensor_copy(k_rep[:, c * NT : (c + 1) * NT], kp[:])

            def prep_q_chunk(c):
                qp = st_pool.tile([96 + D, NT], F32, tag="st", name=f"qp{c}")
                nc.tensor.matmul(
                    qp[:], wq_sb[:], xo_sb[:, c * NT : (c + 1) * NT],
                    start=True, stop=True,
                )
                nc.vector.tensor_copy(q_rep[:, c * NT : (c + 1) * NT], qp[:])

            def prep_vt_quad(p):
                # 4 vT blocks through one PSUM slot, one batched cast
                vp = st_pool.tile([MB, 4 * CA], F32, tag="st", name=f"vp{p}")
                for i in range(4):
                    mb = 4 * p + i
                    nc.tensor.matmul(
                        vp[:, i * CA : (i + 1) * CA],
                        xs_sb[:, mb * MB : (mb + 1) * MB], wv_sb[:],
                        start=True, stop=True,
                    )
                nc.vector.tensor_copy(vT[:, 4 * p * CA : (4 * p + 4) * CA], vp[:])

            prep_k_chunk(0)
            prep_q_chunk(0)

            pending_norm = []
            norm_state = {}

            def norm_a(nt, avA, avB):
                # copy the split accumulators out on DVE and ACT in parallel
                # so both PSUM banks free in one copy-latency, unstalling the
                # next tile's AV matmuls ~700ns earlier
                avAs = o_pool.tile([CA, NT], F32, tag="avAs", name=f"avAs{nt}")
                nc.vector.tensor_copy(avAs[:], avA[:])
                avBs = o_pool.tile([CA, NT], F32, tag="avBs", name=f"avBs{nt}")
                nc.scalar.activation(avBs[:], avB[:], AF.Copy)
                avS = o_pool.tile([CA, NT], F32, tag="avS", name=f"avS{nt}")
                nc.vector.tensor_add(avS[:], avAs[:], avBs[:])
                recip = sm_pool.tile([1, NT], F32, tag="recip", name=f"recip{nt}")
                nc.vector.reciprocal(recip[:], avS[C:CA, :])
                recip_bf = sm_pool.tile([1, NT], BF16, tag="recip_bf", name=f"rb{nt}")
                nc.vector.tensor_copy(recip_bf[:], recip[:])
                norm_state[nt] = (avS, recip_bf)

            def norm_b(nt, avA, avB):
                avS, recip_bf = norm_state.pop(nt)
                n0b, n1b = nt * NT, (nt + 1) * NT
                bc = st_pool.tile([C, NT], F32, tag="st", name=f"bc{nt}")
                nc.tensor.matmul(bc[:], ones_sb[:], recip_bf[:], start=True, stop=True)
                om = o_pool.tile([C, NT], F32, tag="om", name=f"om{nt}")
                nc.vector.tensor_mul(om[:], bc[:], avS[0:C, :])
                o = o_pool.tile([C, NT], F32, tag="o", name=f"o{nt}")
                nc.vector.tensor_add(o[:], om[:], xr_sb[:, n0b:n1b])
                nc.sync.dma_start(out_d[:, n0b:n1b], o[:])

            for nt in range(N_NT):
                n0, n1 = nt * NT, (nt + 1) * NT
                avA = avA_pool.tile([CA, NT], F32, tag="avA", name=f"avA{nt}")
                avB = avB_pool.tile([CA, NT], F32, tag="avB", name=f"avB{nt}")

                def emit_av(e_t, w, avA=avA, avB=avB):
                    for j in range(2):
                        mb = 2 * w + j
                        nc.tensor.matmul(
                            avA[:],
                            vT[0:64, mb * CA : (mb + 1) * CA],
                            e_t[0:64, j * NT : (j + 1) * NT],
                            start=(mb == 0), stop=(mb == N_MB - 1),
                        )
                        nc.tensor.matmul(
                            avB[:],
                            vT[64:MB, mb * CA : (mb + 1) * CA],
                            e_t[64:MB, j * NT : (j + 1) * NT],
                            start=(mb == 0), stop=(mb == N_MB - 1),
                        )

                pend = []
                for p in range(N_MB // 4):  # wave pairs: m-blocks 4p..4p+3
                    # score quad: 4 concurrent rank-8 matmuls in distinct
                    # 32-row PE groups
                    st0 = st_pool.tile([MB, 2 * NT], F32, tag="st", name=f"st{nt}_{p}a")
                    st1 = st_pool.tile([MB, 2 * NT], F32, tag="st", name=f"st{nt}_{p}b")
                    for j4 in range(4):
                        mb = 4 * p + j4
                        rg = 32 * j4
                        stt, col = (st0, j4 * NT) if j4 < 2 else (st1, (j4 - 2) * NT)
                        nc.tensor.matmul(
                            stt[:, col : col + NT],
                            k_rep[rg : rg + D, mb * MB : (mb + 1) * MB],
                            q_rep[rg : rg + D, n0:n1],
                            start=True, stop=True,
                            tile_position=(rg, 0),
                        )
                    e0 = e_pool.tile([MB, 2 * NT], E_DTYPE, tag="e", name=f"e{nt}_{p}a")
                    nc.scalar.activation(e0[:], st0[:], AF.Exp)
                    e1 = e_pool.tile([MB, 2 * NT], E_DTYPE, tag="e", name=f"e{nt}_{p}b")
                    nc.scalar.activation(e1[:], st1[:], AF.Exp)
                    if pending_norm and p == 0:
                        norm_a(*pending_norm[0])
                    if pending_norm and p == 3:
                        norm_b(*pending_norm.pop(0))
                    for args in pend:
                        emit_av(*args)
                    if nt == 0:
                        if p + 1 < 8:
                            prep_k_chunk(p + 1)
                        if p in (1, 3, 5) and p // 2 + 1 < N_NT:
                            prep_q_chunk(p // 2 + 1)
                        prep_vt_quad(p)
                    pend = [(e0, 2 * p), (e1, 2 * p + 1)]
                for args in pend:
                    emit_av(*args)

                pending_norm.append((nt, avA, avB))
                if nt == N_NT - 1:
                    while pending_norm:
                        norm_a(*pending_norm[0])
                        norm_b(*pending_norm.pop(0))
    nc.compile()
    return nc


_NC = None


def _get_nc() -> bass.Bass:
    global _NC
    if _NC is None:
        _NC = build_program()
    return _NC


def _to_bf16(a: np.ndarray) -> np.ndarray:
    """Round-to-nearest-even fp32 -> bf16 (ml_dtypes view)."""
    import ml_dtypes

    u = np.ascontiguousarray(a, np.float32).view(np.uint32)
    rounded = ((u + 0x7FFF + ((u >> 16) & 1)) >> 16).astype(np.uint16)
    return rounded.view(ml_dtypes.bfloat16)


def make_in_maps(x_opt, x_sar, wq, bq, wk, bk, wv, bv, gamma):
    f = np.float32
    x_opt = np.asarray(x_opt, f).reshape(B, C, N)
    x_sar = np.asarray(x_sar, f).reshape(B, C, N)
    g = float(np.asarray(gamma, f).reshape(()))
    wq_aug = np.concatenate([np.asarray(wq, f).T, np.asarray(bq, f)[None, :]], axis=0)
    wk_aug = np.concatenate([np.asarray(wk, f).T, np.asarray(bk, f)[None, :]], axis=0)
    # gamma folded into v (weights AND bias); denominator column stays 1.
    wv_aug = np.zeros((CA, CA), f)
    wv_aug[:C, :C] = np.asarray(wv, f).T * g
    wv_aug[C, :C] = np.asarray(bv, f) * g
    wv_aug[C, C] = 1.0
    wq4 = np.zeros((CA, 96 + D), f)
    wk4 = np.zeros((CA, 96 + D), f)
    for gidx in range(4):
        wq4[:, 32 * gidx : 32 * gidx + D] = wq_aug
        wk4[:, 32 * gidx : 32 * gidx + D] = wk_aug
    wq_bf = _to_bf16(wq4)
    wk_bf = _to_bf16(wk4)
    wv_bf = _to_bf16(wv_aug)
    ones_n = np.ones((1, N), f)
    maps = []
    for core in range(NCORES):
        b, h = divmod(core, 2)
        xo_aug = np.concatenate(
            [x_opt[b, :, h * NL : (h + 1) * NL], ones_n[:, :NL]], axis=0
        )
        xs_aug = np.concatenate([x_sar[b], ones_n], axis=0)
        maps.append(
            {
                "xo_bf": _to_bf16(xo_aug),
                "xs_bf": _to_bf16(xs_aug),
                "xores": np.ascontiguousarray(x_opt[b, :, h * NL : (h + 1) * NL]),
                "wq_bf": wq_bf,
                "wk_bf": wk_bf,
                "wv_bf": wv_bf,
            }
        )
    return maps


def assemble_out(results) -> np.ndarray:
    out = np.empty((B, C, N), np.float32)
    for core in range(NCORES):
        b, h = divmod(core, 2)
        out[b, :, h * NL : (h + 1) * NL] = results[core]["out"]
    return out.reshape(B, C, HH, WW)


def kernel(**inputs) -> np.ndarray:
    nc = _get_nc()
    maps = make_in_maps(**inputs)
    res = run_bass_kernel_spmd(nc, maps, list(range(NCORES)))
    return assemble_out(res.results)


# revision 23
# speedup vs baseline: 1.0332x; 1.0002x over previous
"""Trainium2 Bass kernel for CrossModalFusion (B=4, C=64, H=W=64, N=4096).

Reference computation (per sample b, with x reshaped to [C, N]):
    q = wq @ xo + bq          [8, N]
    k = wk @ xs + bk          [8, N]
    v = wv @ xs + bv          [64, N]
    S[n, m]  = q[:, n] . k[:, m]
    attn     = softmax_m(S)
    out      = gamma * (v @ attn^T) + x_opt

Sharding: 8 cores = 4 batch samples x 2 halves of the query (n) axis.
Each core computes output rows [64, 2048] for its (sample, n-half); no
cross-core communication is needed.

Per-core dataflow (matmuls in bf16 / f32r — the PE in this environment never
leaves the 1.2 GHz throttled clock, so concurrency via PE array tiling is the
main lever):
  - biases are folded into augmented weights on the host (ones-row trick);
    gamma is folded into wv/bv on the host, so the attention output comes out
    pre-scaled and the softmax denominator column stays unscaled.
  - scores are computed TRANSPOSED (S^T[m, n]) so the exp'd scores feed the
    attention*V matmuls directly as the moving operand.  v^T gets an extra
    ones column, so the AV matmuls' row 64 accumulate sum_m exp(S[n, m]) —
    the softmax denominator for free.  No max-subtraction: scores are O(3).
  - q/k are replicated at partition offsets 0/32/64/96 so four rank-8 S^T
    matmuls run concurrently in the four 32-row PE groups.
  - AV matmuls are split into rows 0-63 / 64-127 (two concurrent 64-row PE
    groups) accumulating into separate PSUM tiles avA/avB, summed at
    normalize time.
  - q/k/vT prep is interleaved just-in-time into n-tile 0's wave loop so the
    exp pipeline starts as soon as the first score block exists.
  - per n-tile of 512: accumulate over all 32 m-blocks, normalize by
    1/denominator, add the fp32 x_opt residual, DMA out.
"""

import os
import sys

import numpy as np

for _p in ("/opt/trn_rl_repo", "/root/.axon_site/_ro/trn_rl_repo"):
    if os.path.isdir(_p) and _p not in sys.path:
        sys.path.insert(0, _p)

import concourse.bass as bass
import concourse.mybir as mybir
import concourse.tile as tile
from concourse import bacc
from concourse.bass_utils import run_bass_kernel_spmd

F32 = mybir.dt.float32
F32R = mybir.dt.float32r
BF16 = mybir.dt.bfloat16
AF = mybir.ActivationFunctionType

B, C, HH, WW = 4, 64, 64, 64
N = HH * WW            # 4096 key/query positions
D = 8                  # q/k channel count
CA = C + 1             # augmented channel dim (ones row / denominator row)
NCORES = 8
NL = N // 2            # query rows per core
NT = 512               # n-tile (PSUM bank width in fp32)
MB = 128               # m-block (PE partition width)
N_NT = NL // NT        # 4 n-tiles per core
N_MB = N // MB         # 32 m-blocks
E_DTYPE = F32R         # exp output / AV operand dtype


def build_program(repeat: int = 1) -> bass.Bass:
    nc = bacc.Bacc("TRN2", target_bir_lowering=False, num_devices=NCORES)
    xo_d = nc.declare_dram_parameter("xo_bf", [CA, NL], BF16, isOutput=False)
    xs_d = nc.declare_dram_parameter("xs_bf", [CA, N], BF16, isOutput=False)
    xr_d = nc.declare_dram_parameter("xores", [C, NL], F32, isOutput=False)
    wq_d = nc.declare_dram_parameter("wq_bf", [CA, 96 + D], BF16, isOutput=False)
    wk_d = nc.declare_dram_parameter("wk_bf", [CA, 96 + D], BF16, isOutput=False)
    wv_d = nc.declare_dram_parameter("wv_bf", [CA, CA], BF16, isOutput=False)
    out_d = nc.declare_dram_parameter("out", [C, NL], F32, isOutput=True)

    with tile.TileContext(nc) as tc:
      for _rep in range(repeat):
        with (
            tc.tile_pool(name="const", bufs=1) as cp,
            tc.tile_pool(name="st_ps", bufs=3, space="PSUM") as st_pool,
            tc.tile_pool(name="avA_ps", bufs=1, space="PSUM") as avA_pool,
            tc.tile_pool(name="avB_ps", bufs=1, space="PSUM") as avB_pool,
            tc.tile_pool(name="e_sb", bufs=4) as e_pool,
            tc.tile_pool(name="o_sb", bufs=2) as o_pool,
            tc.tile_pool(name="sm_sb", bufs=2) as sm_pool,
        ):
            xs_sb = cp.tile([CA, N], BF16)
            xo_sb = cp.tile([CA, NL], BF16)
            wk_sb = cp.tile([CA, 96 + D], BF16)
            # startup critical chain: wk -> xs[0:512] (all kp0 needs) ->
            # wq -> xo[0:512] (all qp0 needs), bulk data after
            nc.sync.dma_start(wk_sb[:], wk_d[:])
            nc.sync.dma_start(xs_sb[:, 0:NT], xs_d[:, 0:NT])
            wq_sb = cp.tile([CA, 96 + D], BF16)
            nc.sync.dma_start(wq_sb[:], wq_d[:])
            nc.sync.dma_start(xo_sb[:, 0:NT], xo_d[:, 0:NT])
            wv_sb = cp.tile([CA, CA], BF16)
            nc.sync.dma_start(wv_sb[:], wv_d[:])
            nc.sync.dma_start(xs_sb[:, NT:1024], xs_d[:, NT:1024])
            ones_sb = cp.tile([1, C], BF16)
            nc.vector.memset(ones_sb[:], 1.0)
            nc.sync.dma_start(xo_sb[:, NT:1024], xo_d[:, NT:1024])
            for j in range(1, 4):
                nc.sync.dma_start(
                    xs_sb[:, j * 1024 : (j + 1) * 1024],
                    xs_d[:, j * 1024 : (j + 1) * 1024],
                )
            nc.sync.dma_start(xo_sb[:, 1024:2048], xo_d[:, 1024:2048])
            xr_sb = cp.tile([C, NL], F32)
            nc.sync.dma_start(xr_sb[:], xr_d[:])

            # q/k replicated at partition offsets 0/32/64/96 (score row
            # groups); vT blocks [128, 65] with trailing ones column.
            q_rep = cp.tile([96 + D, NL], BF16)
            k_rep = cp.tile([96 + D, N], BF16)
            vT = cp.tile([MB, N_MB * CA], E_DTYPE)

            # wk_sb/wq_sb hold 4 copies of the weights at col offsets
            # 0/32/64/96, so one matmul lands k/q at all four partition
            # groups and one CAST moves them to SBUF -- no replication DMAs.
            def prep_k_chunk(c):
                kp = st_pool.tile([96 + D, NT], F32, tag="st", name=f"kp{c}")
                nc.tensor.matmul(
                    kp[:], wk_sb[:], xs_sb[:, c * NT : (c + 1) * NT],
                    start=True, stop=True,
                )
                nc.vector.tensor_copy(k_rep[:, c * NT : (c + 1) * NT], kp[:])

            def prep_q_chunk(c):
                qp = st_pool.tile([96 + D, NT], F32, tag="st", name=f"qp{c}")
                nc.tensor.matmul(
                    qp[:], wq_sb[:], xo_sb[:, c * NT : (c + 1) * NT],
                    start=True, stop=True,
                )
                nc.vector.tensor_copy(q_rep[:, c * NT : (c + 1) * NT], qp[:])

            def prep_vt_quad(p):
                # 4 vT blocks through one PSUM slot, one batched cast
                vp = st_pool.tile([MB, 4 * CA], F32, tag="st", name=f"vp{p}")
                for i in range(4):
                    mb = 4 * p + i
                    nc.tensor.matmul(
                        vp[:, i * CA : (i + 1) * CA],
                        xs_sb[:, mb * MB : (mb + 1) * MB], wv_sb[:],
                        start=True, stop=True,
                    )
                nc.vector.tensor_copy(vT[:, 4 * p * CA : (4 * p + 4) * CA], vp[:])

            prep_k_chunk(0)
            prep_q_chunk(0)

            pending_norm = []
            norm_state = {}

            def norm_a(nt, avA, avB):
                # DVE-only half: sum the split accumulators, reciprocal
                avAs = o_pool.tile([CA, NT], F32, tag="avAs", name=f"avAs{nt}")
                nc.vector.tensor_copy(avAs[:], avA[:])
                avS = o_pool.tile([CA, NT], F32, tag="avS", name=f"avS{nt}")
                nc.vector.tensor_add(avS[:], avB[:], avAs[:])
                recip = sm_pool.tile([1, NT], F32, tag="recip", name=f"recip{nt}")
                nc.vector.reciprocal(recip[:], avS[C:CA, :])
                recip_bf = sm_pool.tile([1, NT], BF16, tag="recip_bf", name=f"rb{nt}")
                nc.vector.tensor_copy(recip_bf[:], recip[:])
                norm_state[nt] = (avS, recip_bf)

            def norm_b(nt, avA, avB):
                avS, recip_bf = norm_state.pop(nt)
                n0b, n1b = nt * NT, (nt + 1) * NT
                bc = st_pool.tile([C, NT], F32, tag="st", name=f"bc{nt}")
                nc.tensor.matmul(bc[:], ones_sb[:], recip_bf[:], start=True, stop=True)
                om = o_pool.tile([C, NT], F32, tag="om", name=f"om{nt}")
                nc.vector.tensor_mul(om[:], bc[:], avS[0:C, :])
                o = o_pool.tile([C, NT], F32, tag="o", name=f"o{nt}")
                nc.vector.tensor_add(o[:], om[:], xr_sb[:, n0b:n1b])
                nc.sync.dma_start(out_d[:, n0b:n1b], o[:])

            for nt in range(N_NT):
                n0, n1 = nt * NT, (nt + 1) * NT
                avA = avA_pool.tile([CA, NT], F32, tag="avA", name=f"avA{nt}")
                avB = avB_pool.tile([CA, NT], F32, tag="avB", name=f"avB{nt}")

                def emit_av(e_t, w, avA=avA, avB=avB):
                    for j in range(2):
                        mb = 2 * w + j
                        nc.tensor.matmul(
                            avA[:],
                            vT[0:64, mb * CA : (mb + 1) * CA],
                            e_t[0:64, j * NT : (j + 1) * NT],
                            start=(mb == 0), stop=(mb == N_MB - 1),
                        )
                        nc.tensor.matmul(
                            avB[:],
                            vT[64:MB, mb * CA : (mb + 1) * CA],
                            e_t[64:MB, j * NT : (j + 1) * NT],
                            start=(mb == 0), stop=(mb == N_MB - 1),
                        )

                pend = []
                for p in range(N_MB // 4):  # wave pairs: m-blocks 4p..4p+3
                    # score quad: 4 concurrent rank-8 matmuls in distinct
                    # 32-row PE groups
                    st0 = st_pool.tile([MB, 2 * NT], F32, tag="st", name=f"st{nt}_{p}a")
                    st1 = st_pool.tile([MB, 2 * NT], F32, tag="st", name=f"st{nt}_{p}b")
                    for j4 in range(4):
                        mb = 4 * p + j4
                        rg = 32 * j4
                        stt, col = (st0, j4 * NT) if j4 < 2 else (st1, (j4 - 2) * NT)
                        nc.tensor.matmul(
                            stt[:, col : col + NT],
                            k_rep[rg : rg + D, mb * MB : (mb + 1) * MB],
                            q_rep[rg : rg + D, n0:n1],
                            start=True, stop=True,
                            tile_position=(rg, 0),
                        )
                    e0 = e_pool.tile([MB, 2 * NT], E_DTYPE, tag="e", name=f"e{nt}_{p}a")
                    nc.scalar.activation(e0[:], st0[:], AF.Exp)
                    e1 = e_pool.tile([MB, 2 * NT], E_DTYPE, tag="e", name=f"e{nt}_{p}b")
                    nc.scalar.activation(e1[:], st1[:], AF.Exp)
                    if pending_norm and p == 0:
                        norm_a(*pending_norm[0])
                    if pending_norm and p == 3:
                        norm_b(*pending_norm.pop(0))
                    for args in pend:
                        emit_av(*args)
                    if nt == 0:
                        if p + 1 < 8:
                            prep_k_chunk(p + 1)
                        if p in (1, 3, 5) and p // 2 + 1 < N_NT:
                            prep_q_chunk(p // 2 + 1)
                        prep_vt_quad(p)
                    pend = [(e0, 2 * p), (e1, 2 * p + 1)]
                for args in pend:
                    emit_av(*args)

                pending_norm.append((nt, avA, avB))
                if nt == N_NT - 1:
                    while pending_norm:
                        norm_a(*pending_norm[0])
                        norm_b(*pending_norm.pop(0))
    nc.compile()
    return nc


_NC = None


def _get_nc() -> bass.Bass:
    global _NC
    if _NC is None:
        _NC = build_program()
    return _NC


def _to_bf16(a: np.ndarray) -> np.ndarray:
    """Round-to-nearest-even fp32 -> bf16 (ml_dtypes view)."""
    import ml_dtypes

    u = np.ascontiguousarray(a, np.float32).view(np.uint32)
    rounded = ((u + 0x7FFF + ((u >> 16) & 1)) >> 16).astype(np.uint16)
    return rounded.view(ml_dtypes.bfloat16)


def make_in_maps(x_opt, x_sar, wq, bq, wk, bk, wv, bv, gamma):
    f = np.float32
    x_opt = np.asarray(x_opt, f).reshape(B, C, N)
    x_sar = np.asarray(x_sar, f).reshape(B, C, N)
    g = float(np.asarray(gamma, f).reshape(()))
    wq_aug = np.concatenate([np.asarray(wq, f).T, np.asarray(bq, f)[None, :]], axis=0)
    wk_aug = np.concatenate([np.asarray(wk, f).T, np.asarray(bk, f)[None, :]], axis=0)
    # gamma folded into v (weights AND bias); denominator column stays 1.
    wv_aug = np.zeros((CA, CA), f)
    wv_aug[:C, :C] = np.asarray(wv, f).T * g
    wv_aug[C, :C] = np.asarray(bv, f) * g
    wv_aug[C, C] = 1.0
    wq4 = np.zeros((CA, 96 + D), f)
    wk4 = np.zeros((CA, 96 + D), f)
    for gidx in range(4):
        wq4[:, 32 * gidx : 32 * gidx + D] = wq_aug
        wk4[:, 32 * gidx : 32 * gidx + D] = wk_aug
    wq_bf = _to_bf16(wq4)
    wk_bf = _to_bf16(wk4)
    wv_bf = _to_bf16(wv_aug)
    ones_n = np.ones((1, N), f)
    maps = []
    for core in range(NCORES):
        b, h = divmod(core, 2)
        xo_aug = np.concatenate(
            [x_opt[b, :, h * NL : (h + 1) * NL], ones_n[:, :NL]], axis=0
        )
        xs_aug = np.concatenate([x_sar[b], ones_n], axis=0)
        maps.append(
            {
                "xo_bf": _to_bf16(xo_aug),
                "xs_bf": _to_bf16(xs_aug),
                "xores": np.ascontiguousarray(x_opt[b, :, h * NL : (h + 1) * NL]),
                "wq_bf": wq_bf,
                "wk_bf": wk_bf,
                "wv_bf": wv_bf,
            }
        )
    return maps


def assemble_out(results) -> np.ndarray:
    out = np.empty((B, C, N), np.float32)
    for core in range(NCORES):
        b, h = divmod(core, 2)
        out[b, :, h * NL : (h + 1) * NL] = results[core]["out"]
    return out.reshape(B, C, HH, WW)


def kernel(**inputs) -> np.ndarray:
    nc = _get_nc()
    maps = make_in_maps(**inputs)
    res = run_bass_kernel_spmd(nc, maps, list(range(NCORES)))
    return assemble_out(res.results)
